# revision 41
# baseline (speedup 1.0000x reference)
"""Trainium2 Bass kernel for CombinedLoss (dice + hausdorff), 8-core SPMD.

Sharding: batch B=32 -> 4 samples/core, 12 (b,c) pairs per core.

Device per (b,c), with fp8(e4m3)-quantized inputs:
    P[rb] = x8 @ y8^T - 0.5*y2[j] - 0.5*x2[i]   (fp8 DoubleRow matmuls,
            rank-1 norm terms appended as a 3-way fp8 split, K=3 matmul)
          = -0.5 * d2[i, j]   for i-block rb, all j          (PSUM f32)
    row path: pm[p, rb] = max_j P[rb]            (DVE tensor_tensor_reduce)
    col path: qm2 = max_rb P -> colv[j] = max_p qm2  (ACT cvt + DVE/Pool max,
              GPSIMD partition reduce)
Host: fp8 quantize + transposes + norms; dice term (exact f32 inputs);
final min/max/sqrt/mean combine from pm ([128,48]) and colv ([1,6144]).
"""

import numpy as np
import ml_dtypes

import concourse.bass as bass
import concourse.bacc as bacc
import concourse.mybir as mybir
import concourse.tile as tile
from concourse.bass_utils import run_bass_kernel_spmd
from bass_rust import AxisListType

B, C, H, W = 32, 3, 512, 512
NCORES = 8
BPC = B // NCORES           # samples per core
NBC = BPC * C               # (b,c) pairs per core = 12
WEIGHT_DICE = 0.4
WEIGHT_HAUSDORFF = 0.6
SMOOTH = 1e-05

f32 = mybir.dt.float32
bf16 = mybir.dt.bfloat16
f8 = mybir.dt.float8e4
ALU = mybir.AluOpType
DR = mybir.MatmulPerfMode.DoubleRow
F8NP = ml_dtypes.float8_e4m3
NEG = -3.0e38

_CACHE = {}


def _build(repeat=1):
    nc = bacc.Bacc(None)
    # x8/y8 transposed and w-interleaved: xyt[bc, w, 0] = x8^T[w], [bc, w, 1] = y8^T[w]
    xyt_d = nc.dram_tensor("xyt", [NBC, W, 2, H], f8, kind="ExternalInput")
    # norm-append operands: 3 regions of 512B per partition [h0 | M | O1].
    # k<3:  h0 = ones,  M = k-th split of -0.5*y2, O1 = zeros
    # k>=3: h0 = zeros, M = (k-3)-th split of -0.5*x2 (rb,m), O1 = ones
    # lhsT AP = regions {h0,M} (two-stride 512), rhs AP = regions {M,O1}:
    # the zero blocks kill the cross terms, so one K=6 DoubleRow matmul
    # appends both -0.5*y2[j] and -0.5*x2[i].
    aug_d = nc.dram_tensor("aug", [NBC, 6, 1536], f8, kind="ExternalInput")
    # pm[p, 4*bc+rb] = max_j P[rb]
    # col[16*(bc//4) + 4*(bc%4) + rb, j] = max_p P[rb][p, j]
    pm_d = nc.dram_tensor("pm", [128, 4 * NBC], f32, kind="ExternalOutput")
    col_d = nc.dram_tensor("col", [4 * NBC, H], f32, kind="ExternalOutput")

    with tile.TileContext(nc) as tc:
        with (
            tc.tile_pool(name="const", bufs=1) as cpool,
            tc.tile_pool(name="xy", bufs=3) as xypool,
            tc.tile_pool(name="aug", bufs=4) as augpool,
            tc.tile_pool(name="q", bufs=2) as qpool,
            tc.tile_pool(name="colp", bufs=2) as colpool,
            tc.tile_pool(name="psum", bufs=2, space="PSUM") as ppool,
        ):
            pmres = cpool.tile([128, 4 * NBC], f32, tag="pmres")

            aug_tiles = {}  # bc -> tile, DMA-prefetched 2 iterations ahead

            def fetch_aug(b):
                if b < NBC and b not in aug_tiles:
                    t = augpool.tile([6, 1536], f8, tag="augt", name=f"augt_{b}")
                    nc.scalar.dma_start(t[:], aug_d[b])
                    aug_tiles[b] = t

            for bc in [b for _ in range(repeat) for b in range(NBC)]:
                xyts = xypool.tile([128, 2 * 4 * H], f8, tag="xyts")
                xy5 = xyts[:].rearrange("p (wb t i) -> p wb t i", wb=4, t=2)
                xyv = xyts[:].rearrange("p (wb ti) -> p wb ti", wb=4)
                xysrc = xyt_d[bc].rearrange("(wb p) t i -> p wb (t i)", wb=4)
                if bc == 0:
                    # split the first load across both queues to cut the
                    # time-to-first-matmul roughly in half
                    nc.sync.dma_start(xyv[:, 0:2, :], xysrc[:, 0:2, :])
                    nc.scalar.dma_start(xyv[:, 2:4, :], xysrc[:, 2:4, :])
                else:
                    nc.sync.dma_start(xyv, xysrc)
                # norm-append operands, prefetched 2 bcs ahead on the ACT
                # queue so appends never wait on the aug DMA
                fetch_aug(bc)
                fetch_aug(bc + 1)
                fetch_aug(bc + 2)
                augt = aug_tiles[bc]
                a3 = augt[:].rearrange("k (s x) -> k s x", s=3)
                ar = a3[:, 1:3, :]  # rhs: regions {M, O1}

                g, seg = divmod(bc, 4)
                if seg == 0:
                    coltile = colpool.tile([1, 16 * H], f32, tag="coltile",
                                           name=f"colt_{g}")
                cofs = 2048 * seg

                def mm_group(P, rb, app_first):
                    # norm append opens the group: it only needs the
                    # (prefetched) aug tile, so it never delays the tile.
                    # (bc0: the aug lands after the split xyt halves, so
                    # open with the mains there instead.)
                    lhsT = a3[:, 0:2, 128 * rb : 128 * rb + 128]
                    if app_first:
                        nc.tensor.matmul(
                            P, lhsT, ar, start=True, stop=False, perf_mode=DR
                        )
                    for u in range(2):
                        nc.tensor.matmul(
                            P,
                            xy5[:, 2 * u : 2 * u + 2, 0, 128 * rb : 128 * rb + 128],
                            xy5[:, 2 * u : 2 * u + 2, 1, :],
                            start=(u == 0 and not app_first),
                            stop=(u == 1 and app_first),
                            perf_mode=DR,
                        )
                    if not app_first:
                        nc.tensor.matmul(
                            P, lhsT, ar, start=False, stop=True, perf_mode=DR
                        )

                s23 = qpool.tile([128, 512], bf16, tag="s23")
                # tiles 0,1: single-bank PSUM; DVE ttr-full = bf16 egress
                # (for the col path) fused with the row j-reduce
                for rb in range(2):
                    P = ppool.tile([128, H], f32, tag=f"P{rb}", name=f"P{rb}_{bc}")
                    mm_group(P[:], rb, app_first=(bc != 0))
                    q = qpool.tile([128, H], bf16, tag=f"q{rb}", name=f"q{rb}_{bc}")
                    nc.vector.tensor_tensor_reduce(
                        out=q[:],
                        in0=P[:],
                        in1=P[:],
                        scale=1.0,
                        scalar=NEG,
                        op0=ALU.max,
                        op1=ALU.max,
                        accum_out=pmres[:, 4 * bc + rb : 4 * bc + rb + 1],
                    )
                    # col path: partition max from the bf16 copy (GPSIMD)
                    nc.gpsimd.tensor_reduce(
                        coltile[:, cofs + 512 * rb : cofs + 512 * rb + 512],
                        q[:],
                        axis=AxisListType.C,
                        op=ALU.max,
                    )
                # tiles 2,3: two-bank PSUM pair; ACT copies both to bf16,
                # Pool reduces the pair, DVE does the two cheap bf16 rows
                P23 = ppool.tile([128, 2 * H], f32, tag="P23", name=f"P23_{bc}")
                for r in range(2):
                    mm_group(P23[:, 512 * r : 512 * r + 512], 2 + r,
                             app_first=(bc != 0))
                q23 = qpool.tile([128, 2 * H], bf16, tag="q23")
                nc.scalar.copy(q23[:], P23[:])
                nc.gpsimd.tensor_reduce(
                    coltile[:, cofs + 1024 : cofs + 2048],
                    q23[:],
                    axis=AxisListType.C,
                    op=ALU.max,
                )
                for r in range(2):
                    nc.vector.tensor_tensor_reduce(
                        out=s23[:, 256 * r : 256 * r + 256],
                        in0=q23[:, 512 * r : 512 * r + 256],
                        in1=q23[:, 512 * r + 256 : 512 * r + 512],
                        scale=1.0,
                        scalar=NEG,
                        op0=ALU.max,
                        op1=ALU.max,
                        accum_out=pmres[:, 4 * bc + 2 + r : 4 * bc + 3 + r],
                    )
                if seg == 3:
                    # ship 4 bcs of col partials in one DMA on the ACT queue
                    nc.scalar.dma_start(
                        col_d[16 * g : 16 * g + 16, :], coltile[:]
                    )
            nc.sync.dma_start(pm_d[:], pmres[:])
    nc.finalize()
    return nc


def _split3(v):
    """3-term fp8(e4m3) split of v: s1+s2+s3 ~= v to ~2^-12 relative."""
    s1 = v.astype(F8NP)
    r = v - s1.astype(np.float32)
    s2 = r.astype(F8NP)
    r -= s2.astype(np.float32)
    s3 = r.astype(F8NP)
    return s1, s2, s3


def kernel(input, target, _stats=None):
    x = np.asarray(input, dtype=np.float32)
    y = np.asarray(target, dtype=np.float32)

    # ---- host: dice term (exact f32 inputs) ----
    xf = x.reshape(B, -1).astype(np.float64)
    yf = y.reshape(B, -1).astype(np.float64)
    inter = (xf * yf).sum(axis=1)
    union = xf.sum(axis=1) + yf.sum(axis=1)
    dice = float(np.mean(1.0 - (2.0 * inter + SMOOTH) / (union + SMOOTH)))

    # ---- host: fp8 quantize + layout prep ----
    x8 = x.astype(F8NP)
    y8 = y.astype(F8NP)
    xq = x8.astype(np.float64)
    yq = y8.astype(np.float64)
    x2 = (xq * xq).sum(axis=-1).astype(np.float32)  # [B,C,H] norms of quantized pts
    y2 = (yq * yq).sum(axis=-1).astype(np.float32)
    xt8 = np.ascontiguousarray(x8.transpose(0, 1, 3, 2))  # [B,C,W,H] fp8
    yt8 = np.ascontiguousarray(y8.transpose(0, 1, 3, 2))

    in_maps = []
    for c in range(NCORES):
        b0 = c * BPC
        # [NBC, W, 2, H]: x^T and y^T interleaved per w row
        xyt = np.stack(
            [
                xt8[b0 : b0 + BPC].reshape(NBC, W, H),
                yt8[b0 : b0 + BPC].reshape(NBC, W, H),
            ],
            axis=2,
        )
        # aug regions per partition: [h0 | M | O1] (512B each, see _build)
        xs = _split3((-0.5 * x2[b0 : b0 + BPC]).reshape(NBC, 4 * 128))
        ys = _split3((-0.5 * y2[b0 : b0 + BPC]).reshape(NBC, H))
        aug = np.zeros((NBC, 6, 3, 512), dtype=F8NP)
        for k in range(3):
            aug[:, k, 0, :] = np.float32(1.0)  # h0 = ones (y2 rows)
            aug[:, k, 1, :] = ys[k]            # M  = y2 split
            aug[:, 3 + k, 1, :] = xs[k]        # M  = x2 split (rb,m)
            aug[:, 3 + k, 2, :] = np.float32(1.0)  # O1 = ones (x2 rows)
        in_maps.append(
            {
                "xyt": np.ascontiguousarray(xyt),
                "aug": np.ascontiguousarray(aug.reshape(NBC, 6, 1536)),
            }
        )

    if "nc" not in _CACHE:
        _CACHE["nc"] = _build()
    nc = _CACHE["nc"]

    import time as _time

    t0 = _time.time()
    br = run_bass_kernel_spmd(nc, in_maps, list(range(NCORES)), trace=False)
    t1 = _time.time()
    if isinstance(_stats, dict):
        _stats["wall_s"] = t1 - t0
        reps = _stats.get("repeats", 0)
        times = []
        for _ in range(reps):
            ta = _time.time()
            br = run_bass_kernel_spmd(nc, in_maps, list(range(NCORES)), trace=False)
            times.append(_time.time() - ta)
        _stats["repeat_wall_s"] = times

    # ---- host: combine ----
    hds = []
    for c in range(NCORES):
        pm = np.asarray(br.results[c]["pm"], dtype=np.float32)  # [128, 48]
        colv = np.asarray(br.results[c]["col"], dtype=np.float32).reshape(
            NBC, 4, H
        )  # per-bc col partials by i-block
        for bc in range(NBC):
            dxy2 = -2.0 * float(pm[:, 4 * bc : 4 * bc + 4].min())
            dyx2 = -2.0 * float(colv[bc].max(axis=0).min())
            hds.append(np.sqrt(max(dxy2, dyx2, 0.0)))
    hd = float(np.mean(hds))

    loss = WEIGHT_DICE * dice + WEIGHT_HAUSDORFF * hd
    return np.float32(loss)
